# revision 19
# baseline (speedup 1.0000x reference)
"""Trainium2 Bass kernel for LoFTR-style encoder layer (sparse attention + convs).

Sharding: pure data-parallel over batch B=8 -> 8 NeuronCores (one batch
element per core). BN statistics are all-reduced across cores.

Device layout is channel-major; the host pre-pads/casts feat0 to bf16
[128, 2, 84, 82] (conv-padded, image at rows 2..81, cols 1..80) so the
kernel DMAs straight into the conv1 input tiles.  K/V projections read
128-position blocks of the padded flat layout; pad positions project to
zero, giving K rows of exactly 1.0 (elu(0)+1) whose only side effect is
a constant +80 on Ksum, subtracted when building the block-diag matrix.

Emission follows the dependency chain (K/V proj -> KV -> Q -> message ->
conv1 -> BN1 -> conv2 -> BN2 -> out); the tile scheduler fills attention
stalls with independent conv1 matmuls (separate PSUM pools per concern).
"""

import os
import sys

import numpy as np

for _p in ("/opt/trn_rl_repo", os.path.expanduser("~/.axon_site/_ro/trn_rl_repo")):
    if os.path.isdir(_p) and _p not in sys.path:
        sys.path.insert(0, _p)

import ml_dtypes

import concourse.bass as bass
import concourse.mybir as mybir
import concourse.tile as tile
from concourse import bacc
from concourse.bass_utils import run_bass_kernel_spmd

F32 = mybir.dt.float32
BF16 = mybir.dt.bfloat16
AF = mybir.ActivationFunctionType
ALU = mybir.AluOpType

NCORES = 8
H = W = 80
HW = H * W          # 6400
D = 256
NI = 3200           # inside positions (image rows 0..39)
NHEAD = 8
PW = W + 2          # 82 padded width
ATTN_EPS = 1e-6
BN_EPS = 1e-5
BN_N = float(NCORES * HW)

# conv row-tiling: 5 output rows per psum tile -> N = 5*82 = 410 <= 512
RT = 5
NRT = H // RT       # 16
NT = RT * PW        # 410

# inside region in padded-flat coords: tile rows 2..41 -> [164, 3444)
IN0 = 2 * PW        # 164
NKV = 26            # ceil(3280 / 128); last tile is 80 wide
NPAD_IN = 80        # pad slots inside [164, 3444): 2 per row * 40 rows

# remote-dma 1-hop allreduce: correct by construction but the build-time
# scheduling sim is single-core and deadlocks on the remote semaphores,
# so it stays disabled until the sim models cross-core increments
FAST_AR = os.environ.get("FAST_AR", "0") == "1"

LAST_EXEC_NS = None
LAST_MEAN_EXEC_NS = None

_cache = {}


def _bd(ap3):
    return ap3.rearrange("p a b -> p (a b)")


def _r3(ap2, a):
    return ap2.rearrange("p (a b) -> p a b", a=a)


def build_nc():
    nc = bacc.Bacc(
        "TRN2", target_bir_lowering=False, debug=False, num_devices=NCORES
    )

    ip1_d = nc.dram_tensor("ip1f", [128, 2, 84, PW], BF16, kind="ExternalInput")
    wqt_d = nc.dram_tensor("wqt", [128, 2, D], BF16, kind="ExternalInput")
    wkv_d = nc.dram_tensor("wkv", [128, 2, 2 * D], BF16, kind="ExternalInput")
    c1w_d = nc.dram_tensor("c1w", [128, 36, D], BF16, kind="ExternalInput")
    c2w_d = nc.dram_tensor("c2w", [128, 18, D], BF16, kind="ExternalInput")
    bn1g_d = nc.dram_tensor("bn1g", [D, 1], F32, kind="ExternalInput")
    bn1b_d = nc.dram_tensor("bn1b", [D, 1], F32, kind="ExternalInput")
    bn2g_d = nc.dram_tensor("bn2g", [D, 1], F32, kind="ExternalInput")
    bn2b_d = nc.dram_tensor("bn2b", [D, 1], F32, kind="ExternalInput")
    mblk_d = nc.dram_tensor("mblk", [8, 256], BF16, kind="ExternalInput")
    out_d = nc.dram_tensor("out_t", [D, HW], F32, kind="ExternalOutput")

    groups = [list(range(NCORES))]

    with tile.TileContext(nc) as tc:
        with (
            tc.tile_pool(name="pers", bufs=1) as pers,
            tc.tile_pool(name="bigp", bufs=2) as bigp,
            tc.tile_pool(name="qtp", bufs=1) as qtp,
            tc.tile_pool(name="scr", bufs=7) as scr,
            tc.tile_pool(name="sqp", bufs=3) as sqp,
            tc.tile_pool(name="rsp", bufs=5) as rsp,
            tc.tile_pool(name="small", bufs=1) as small,
            tc.tile_pool(name="fin", bufs=5) as fin,
            tc.tile_pool(name="sqF", bufs=2) as sqF,
            tc.tile_pool(name="psA", bufs=3, space="PSUM") as psA,
            tc.tile_pool(name="psK", bufs=2, space="PSUM") as psK,
            tc.tile_pool(name="psC", bufs=2, space="PSUM") as psC,
            tc.tile_pool(name="psF", bufs=1, space="PSUM") as psF,
            tc.tile_pool(name="dram", bufs=1, space="DRAM") as dramp,
        ):
            if FAST_AR:
                ar_rsem = nc.alloc_semaphore("ar_rsem")
                ar_lsem = nc.alloc_semaphore("ar_lsem")
                gbuf = [
                    pers.tile([128, 8, 4], F32, tag=f"gbuf{t}", name=f"gbuf{t}")
                    for t in range(2)
                ]
                for t in range(2):
                    nc.vector.memset(gbuf[t][:, :, :], 0.0)
                nc.gpsimd.bir_kernel_barrier_wait(groups)
            # ---------------- weights / inputs (chunked for early start) ----
            wqt = pers.tile([128, 2, D], BF16, tag="wqt", name="wqt")
            wkv = pers.tile([128, 2, 2 * D], BF16, tag="wkv", name="wkv")
            c1w = pers.tile([128, 36, D], BF16, tag="c1w", name="c1w")
            c2w = pers.tile([128, 18, D], BF16, tag="c2w", name="c2w")
            ip1 = [
                pers.tile([128, 84, PW], BF16, tag=f"ip1_{c}", name=f"ip1_{c}")
                for c in range(4)
            ]
            ip2 = [
                pers.tile([128, 84, PW], BF16, tag=f"ip2_{c}", name=f"ip2_{c}")
                for c in range(2)
            ]
            maskblk = pers.tile([8, 256], BF16, tag="maskblk", name="maskblk")

            nc.sync.dma_start(wkv[:, :, :], wkv_d[:, :, :])
            # f-halves of the padded input; tiny first chunk so the first
            # K/V + conv1 matmuls start as early as possible
            for r0, r1 in ((0, 7), (7, 21), (21, 42), (42, 63), (63, 84)):
                for m in range(2):
                    nc.sync.dma_start(
                        ip1[m][:, r0:r1, :], ip1_d[:, m, r0:r1, :]
                    )
                if r0 == 0:
                    nc.sync.dma_start(c1w[:, 0:9, :], c1w_d[:, 0:9, :])
                    nc.sync.dma_start(c1w[:, 9:18, :], c1w_d[:, 9:18, :])
                if r0 == 21:
                    nc.sync.dma_start(c1w[:, 18:36, :], c1w_d[:, 18:36, :])
                if r0 == 42:
                    nc.sync.dma_start(wqt[:, :, :], wqt_d[:, :, :])
            nc.sync.dma_start(c2w[:, :, :], c2w_d[:, :, :])
            nc.sync.dma_start(maskblk[:, :], mblk_d[:, :])

            eps_t = small.tile([128, 1], F32, tag="eps_t", name="eps_t")
            nc.vector.memset(eps_t[:, :], BN_EPS)
            g1 = small.tile([128, 2], F32, tag="g1", name="g1")
            b1 = small.tile([128, 2], F32, tag="b1", name="b1")
            g2 = small.tile([128, 2], F32, tag="g2", name="g2")
            b2 = small.tile([128, 2], F32, tag="b2", name="b2")
            for o in range(2):
                sl = slice(o * 128, (o + 1) * 128)
                nc.sync.dma_start(g1[:, o : o + 1], bn1g_d[sl, :])
                nc.sync.dma_start(b1[:, o : o + 1], bn1b_d[sl, :])
                nc.sync.dma_start(g2[:, o : o + 1], bn2g_d[sl, :])
                nc.sync.dma_start(b2[:, o : o + 1], bn2b_d[sl, :])

            # ---------------- zero-init --------------------------------------
            # (gpsimd sits behind the prelude barrier when FAST_AR, so early
            #  needs go on DVE; ip2 pads aren't read until conv2)
            ke = bigp.tile([128, NKV, D], BF16, tag="big", name="ke")
            ve = bigp.tile([128, NKV, D + 1], BF16, tag="big", name="ve")
            nc.vector.memset(ve[:, :, :], 1.0)  # col 0 of each chunk = ones
            for c in (2, 3):
                # t-channels: zero everything except where message is scattered
                nc.vector.memset(ip1[c][:, 0:42, :], 0.0)
                nc.vector.memset(ip1[c][:, 82:84, :], 0.0)
                nc.vector.memset(ip1[c][:, 42:82, 0:1], 0.0)
                nc.vector.memset(ip1[c][:, 42:82, 81:82], 0.0)
            for c in range(2):
                # ip2 pads (interior is fully written by conv1)
                nc.vector.memset(ip2[c][:, 0:2, :], 0.0)
                nc.vector.memset(ip2[c][:, 82:84, :], 0.0)
                nc.vector.memset(ip2[c][:, 2:82, 0:1], 0.0)
                nc.vector.memset(ip2[c][:, 2:82, 81:82], 0.0)

            ip1f = [_bd(ip1[c][:, :, :]) for c in range(4)]
            ip2f = [_bd(ip2[c][:, :, :]) for c in range(2)]

            # ---------------- K / V projections ([s, c] layout) -------------
            # one packed matmul per (i, ki): [128pos, 512] = [K | V]
            for i in range(NKV):
                mw = 128 if i < NKV - 1 else 80
                ps = psA.tile([128, 2 * D], F32, tag="psA", name="psA")
                for ki in range(2):
                    nc.tensor.matmul(
                        ps[:mw, :],
                        ip1f[ki][:, IN0 + i * 128 : IN0 + i * 128 + mw],
                        wkv[:, ki, :],
                        start=(ki == 0),
                        stop=(ki == 1),
                    )
                # elu(x)+1 = relu(x) + exp(min(x,0))
                sm = scr.tile([128, 2 * D], F32, tag="scr", name="sm")
                se = scr.tile([128, 2 * D], F32, tag="scr", name="se")
                nc.vector.tensor_scalar_min(sm[:mw, :D], ps[:mw, :D], 0.0)
                nc.scalar.activation(se[:mw, :D], sm[:mw, :D], AF.Exp)
                nc.vector.scalar_tensor_tensor(
                    ke[:mw, i, :], ps[:mw, :D], 0.0, se[:mw, :D], ALU.max, ALU.add
                )
                nc.scalar.copy(ve[:mw, i, 1:], ps[:mw, D:])

            # ---------------- KV + Ksum -> block-diag BD --------------------
            bd = [
                pers.tile([128, 264], BF16, tag=f"bd{m}", name=f"bd{m}")
                for m in range(2)
            ]
            for m in range(2):
                psm = psK.tile([128, D + 1], F32, tag="psK", name="psK")
                for i in range(NKV):
                    mw = 128 if i < NKV - 1 else 80
                    nc.tensor.matmul(
                        psm[:, :],
                        ke[:mw, i, m * 128 : (m + 1) * 128],
                        ve[:mw, i, :],
                        start=(i == 0),
                        stop=(i == NKV - 1),
                    )
                nc.vector.memset(bd[m][:, :], 0.0)
                for hh in range(4):
                    h = m * 4 + hh
                    lh = hh * 32
                    nc.vector.tensor_copy(
                        bd[m][lh : lh + 32, h * 32 : (h + 1) * 32],
                        psm[lh : lh + 32, 1 + h * 32 : 1 + (h + 1) * 32],
                    )
                    # pad positions add exactly NPAD_IN ones to Ksum
                    nc.vector.tensor_scalar_add(
                        bd[m][lh : lh + 32, 256 + h : 257 + h],
                        psm[lh : lh + 32, 0:1],
                        -float(NPAD_IN),
                    )

            # ---------------- Q projections + elu (all l-tiles) -------------
            qt = [
                [
                    qtp.tile([128, NT], BF16, tag=f"qt{q}_{m}", name=f"qt{q}_{m}")
                    for m in range(2)
                ]
                for q in range(8)
            ]
            # per q: projections, elu, S = Ksum.Q, Z = 1/(S+eps) — fused so the
            # Z chain's DVE ops sit right behind the elu in queue order
            rs = []
            for q in range(8):
                lr = 42 + 5 * q
                for m in range(2):
                    ps = psA.tile([128, 2 * D], F32, tag="psA", name="psA")
                    for ki in range(2):
                        nc.tensor.matmul(
                            ps[:, :NT],
                            wqt[:, ki, m * 128 : (m + 1) * 128],
                            ip1f[ki][:, lr * PW : lr * PW + NT],
                            start=(ki == 0),
                            stop=(ki == 1),
                        )
                    sm = scr.tile([128, 2 * D], F32, tag="scr", name="smq")
                    se = scr.tile([128, 2 * D], F32, tag="scr", name="seq")
                    nc.vector.tensor_scalar_min(sm[:, :NT], ps[:, :NT], 0.0)
                    nc.scalar.activation(se[:, :NT], sm[:, :NT], AF.Exp)
                    nc.vector.scalar_tensor_tensor(
                        qt[q][m][:, :], ps[:, :NT], 0.0, se[:, :NT],
                        ALU.max, ALU.add,
                    )
                pss = psK.tile([8, NT], F32, tag="psK", name="psS")
                for ki in range(2):
                    nc.tensor.matmul(
                        pss[:, :],
                        bd[ki][:, 256:264],
                        qt[q][ki][:, :],
                        start=(ki == 0),
                        stop=(ki == 1),
                    )
                sadd = rsp.tile([8, NT], F32, tag="sadd", name="sadd")
                r = rsp.tile([8, NT], BF16, tag="rs", name="rs")
                nc.vector.tensor_scalar_add(sadd[:, :], pss[:, :], ATTN_EPS)
                with nc.allow_low_precision(reason="Z recip feeds bf16 conv"):
                    nc.vector.reciprocal(r[:, :], sadd[:, :])
                rs.append(r)

            # ---------- conv1 helper ----------------------------------------
            stats1 = small.tile([128, 4 * NRT], F32, tag="stats1", name="stats1")
            stats2 = small.tile([128, 4 * NRT], F32, tag="stats2", name="stats2")

            def conv1_tile(j, pool, ptag, sqpool, sqtag):
                r0 = RT * j
                for o in range(2):
                    taps = [(c, k) for c in range(2) for k in range(9)]
                    for c in (2, 3):
                        for ky in range(3):
                            if ky < 37 - r0:
                                continue  # t rows all zero in this stream
                            for kx in range(3):
                                taps.append((c, ky * 3 + kx))
                    ps = pool.tile([128, NT], F32, tag=ptag, name="psc1")
                    for idx, (c, k) in enumerate(taps):
                        ky, kx = divmod(k, 3)
                        s = (r0 + ky + 1) * PW + kx - 1
                        nc.tensor.matmul(
                            ps[:, :],
                            c1w[:, c * 9 + k, o * 128 : (o + 1) * 128],
                            ip1f[c][:, s : s + NT],
                            start=(idx == 0),
                            stop=(idx == len(taps) - 1),
                        )
                    val = _r3(ps[:, :], RT)[:, :, 1:81]
                    # copy raw y1 into ip2 interior + position-sum, on ACT
                    nc.scalar.activation(
                        ip2[o][:, r0 + 2 : r0 + 7, 1:81], val, AF.Identity,
                        accum_out=stats1[:, o * NRT + j : o * NRT + j + 1],
                    )
                    sq = sqpool.tile([128, NT], BF16, tag=sqtag, name="sq")
                    nc.scalar.activation(
                        _r3(sq[:, :], RT)[:, :, 1:81], val, AF.Square,
                        accum_out=stats1[:, (2 + o) * NRT + j :
                                         (2 + o) * NRT + j + 1],
                    )

            # ---------------- message + scatter, then dependent conv1 -------
            for q in range(8):
                for m in range(2):
                    psg = psA.tile([128, 2 * D], F32, tag="psA", name="psA")
                    for ki in range(2):
                        nc.tensor.matmul(
                            psg[:, :NT],
                            bd[ki][:, m * 128 : (m + 1) * 128],
                            qt[q][ki][:, :],
                            start=(ki == 0),
                            stop=(ki == 1),
                        )
                    pre = psA.tile([128, 2 * D], F32, tag="psA", name="psA")
                    nc.tensor.matmul(
                        pre[:, :NT], maskblk[:, m * 128 : (m + 1) * 128],
                        rs[q][:, :],
                    )
                    preb = scr.tile([128, 2 * D], F32, tag="scr", name="preb")
                    nc.vector.tensor_copy(preb[:, :NT], pre[:, :NT])
                    nc.vector.tensor_tensor(
                        ip1[2 + m][:, 42 + 5 * q : 47 + 5 * q, 1:81],
                        _r3(psg[:, :NT], RT)[:, :, 1:81],
                        _r3(preb[:, :NT], RT)[:, :, 1:81],
                        ALU.mult,
                    )
                conv1_tile(q + 7, psC, "psC", sqp, "sq")
            conv1_tile(15, psC, "psC", sqp, "sq")

            # ---------------- independent conv1 (f-channels only) -----------
            # emitted last in the phase (lowest priority) with private PSUM /
            # scratch pools: the scheduler pulls these forward into every
            # attention stall without rotation edges back into the msg chain
            for j in range(7):
                conv1_tile(j, psF, "psF", sqF, "sqF")

            # ---------------- BN allreduce (2KB, latency-critical) ----------
            ar_round = [0]

            def bn_allreduce(stats, tag):
                bnst = small.tile([128, 4], F32, tag=f"bnst{tag}", name=f"bnst{tag}")
                nc.vector.tensor_reduce(
                    bnst[:, :], stats[:, :].rearrange("p (k j) -> p k j", j=NRT),
                    mybir.AxisListType.X, ALU.add,
                )
                gst = small.tile([128, 4], F32, tag=f"gst{tag}", name=f"gst{tag}")
                if FAST_AR:
                    # 1-hop all-to-all over the on-chip torus: XOR-distance i
                    # lands in gather slot i on every core (SPMD-safe), then a
                    # local tree-add.  remote_sem += 2 per arrived transfer.
                    rnd = ar_round[0]
                    ar_round[0] += 1
                    gb = gbuf[rnd]
                    for dist in range(8):
                        rd = [None] * 8
                        rd[dist] = (0, dist)
                        nc.gpsimd.remote_dma_broadcast(
                            gb[:, dist, :], bnst[:, :], ar_rsem, ar_lsem,
                            rdests=rd,
                        )
                    nc.gpsimd.trigger_dma(count=None)
                    nc.vector.wait_ge(ar_rsem, 16 * (rnd + 1))
                    h1 = small.tile([128, 4, 4], F32, tag=f"arh1{tag}",
                                    name=f"arh1{tag}")
                    h2 = small.tile([128, 2, 4], F32, tag=f"arh2{tag}",
                                    name=f"arh2{tag}")
                    nc.vector.tensor_tensor(
                        h1[:, :, :], gb[:, 0:4, :], gb[:, 4:8, :], ALU.add
                    )
                    nc.vector.tensor_tensor(
                        h2[:, :, :], h1[:, 0:2, :], h1[:, 2:4, :], ALU.add
                    )
                    nc.vector.tensor_tensor(
                        gst[:, :], h2[:, 0, :], h2[:, 1, :], ALU.add
                    )
                    return gst
                arin = dramp.tile([128, 4], F32, tag=f"arin{tag}", name=f"arin{tag}")
                arout = dramp.tile([128, 4], F32, tag=f"arout{tag}", name=f"arout{tag}")
                nc.sync.dma_start(arin[:, :], bnst[:, :])
                nc.gpsimd.collective_compute(
                    "AllReduce", ALU.add, replica_groups=groups,
                    ins=[arin[:, :].opt()], outs=[arout[:, :].opt()],
                )
                nc.sync.dma_start(gst[:, :], arout[:, :])
                return gst

            def bn_coeffs(gst, gg, bb, tag):
                # gst = [sum_o0, sum_o1, sq_o0, sq_o1] -> both halves at once
                nm = small.tile([128, 2], F32, tag=f"nm{tag}", name=f"nm{tag}")
                ex2 = small.tile([128, 2], F32, tag=f"ex2{tag}", name=f"ex2{tag}")
                m2 = small.tile([128, 2], F32, tag=f"m2{tag}", name=f"m2{tag}")
                var = small.tile([128, 2], F32, tag=f"var{tag}", name=f"var{tag}")
                sd = small.tile([128, 2], F32, tag=f"sd{tag}", name=f"sd{tag}")
                rsd = small.tile([128, 2], F32, tag=f"rsd{tag}", name=f"rsd{tag}")
                scl = small.tile([128, 2], F32, tag=f"scl{tag}", name=f"scl{tag}")
                sht = small.tile([128, 2], F32, tag=f"sht{tag}", name=f"sht{tag}")
                sh = small.tile([128, 2], F32, tag=f"sh{tag}", name=f"sh{tag}")
                nc.vector.tensor_scalar_mul(nm[:, :], gst[:, 0:2], -1.0 / BN_N)
                nc.vector.tensor_scalar_mul(ex2[:, :], gst[:, 2:4], 1.0 / BN_N)
                # var_neg = m^2 - E[x^2];  sd = sqrt(-var_neg + eps)
                nc.vector.tensor_tensor(m2[:, :], nm[:, :], nm[:, :], ALU.mult)
                nc.vector.tensor_tensor(
                    var[:, :], m2[:, :], ex2[:, :], ALU.subtract
                )
                nc.scalar.activation(
                    sd[:, :], var[:, :], AF.Sqrt, bias=eps_t[:, 0:1], scale=-1.0
                )
                nc.vector.reciprocal(rsd[:, :], sd[:, :])
                nc.vector.tensor_tensor(scl[:, :], rsd[:, :], gg[:, :], ALU.mult)
                nc.vector.tensor_tensor(sht[:, :], nm[:, :], scl[:, :], ALU.mult)
                nc.vector.tensor_tensor(sh[:, :], sht[:, :], bb[:, :], ALU.add)
                return scl, sh

            gst1 = bn_allreduce(stats1, "1")
            scl1, sh1 = bn_coeffs(gst1, g1, b1, "1")

            # ---------------- normalize (in-place) pipelined with conv2 -----
            y2 = [
                bigp.tile([128, H, W], BF16, tag="big", name=f"y2_{o}")
                for o in range(2)
            ]

            def norm_chunk(ra, rb):
                for o in range(2):
                    nc.vector.tensor_scalar(
                        ip2[o][:, ra:rb, 1:81],
                        ip2[o][:, ra:rb, 1:81],
                        scl1[:, o : o + 1],
                        sh1[:, o : o + 1],
                        ALU.mult,
                        ALU.add,
                    )

            def conv2_tile(j):
                r0 = RT * j
                for o in range(2):
                    ps = psC.tile([128, NT], F32, tag="psC", name="psC")
                    idx = 0
                    for c in range(2):
                        for k in range(9):
                            ky, kx = divmod(k, 3)
                            s = (r0 + ky + 1) * PW + kx - 1
                            nc.tensor.matmul(
                                ps[:, :],
                                c2w[:, c * 9 + k, o * 128 : (o + 1) * 128],
                                ip2f[c][:, s : s + NT],
                                start=(idx == 0),
                                stop=(idx == 17),
                            )
                            idx += 1
                    val = _r3(ps[:, :], RT)[:, :, 1:81]
                    nc.scalar.activation(
                        y2[o][:, r0 : r0 + 5, :], val, AF.Identity,
                        accum_out=stats2[:, o * NRT + j : o * NRT + j + 1],
                    )
                    sq = sqp.tile([128, NT], BF16, tag="sq", name="sq2")
                    nc.scalar.activation(
                        _r3(sq[:, :], RT)[:, :, 1:81], val, AF.Square,
                        accum_out=stats2[:, (2 + o) * NRT + j :
                                         (2 + o) * NRT + j + 1],
                    )

            norm_chunk(2, 8)
            conv2_tile(0)
            for k in range(1, 8):
                norm_chunk(10 * k - 2, 10 * k + 8)
                conv2_tile(2 * k - 1)
                conv2_tile(2 * k)
            norm_chunk(78, 82)
            conv2_tile(15)

            # ---------------- BN2 allreduce + residual + store --------------
            gst2 = bn_allreduce(stats2, "2")
            scl2, sh2 = bn_coeffs(gst2, g2, b2, "2")

            for o in range(2):
                for k in range(8):
                    n = o * 8 + k
                    fsl = slice(800 * k, 800 * (k + 1))
                    tmp = fin.tile([128, 800], F32, tag="tmp", name="tmp")
                    nc.scalar.activation(
                        _r3(tmp[:, :], 10),
                        y2[o][:, 10 * k : 10 * (k + 1), :],
                        AF.Identity,
                        bias=sh2[:, o : o + 1],
                        scale=scl2[:, o : o + 1],
                    )
                    eng = nc.gpsimd if n % 3 == 2 else nc.vector
                    eng.tensor_tensor(
                        _r3(tmp[:, :], 10),
                        _r3(tmp[:, :], 10),
                        ip1[o][:, 2 + 10 * k : 12 + 10 * k, 1:81],
                        ALU.add,
                    )
                    nc.sync.dma_start(out_d[o * 128 : (o + 1) * 128, fsl], tmp[:, :])

    nc.compile()
    return nc


def _mblk():
    mb = np.zeros((8, 256), np.float32)
    for h in range(8):
        mb[h, h * 32 : (h + 1) * 32] = 1.0
    return mb.astype(ml_dtypes.bfloat16)


def _prep_inputs(feat0, zone_mask, w_q, w_k, w_v, conv1_w, bn1_g, bn1_b,
                 conv2_w, bn2_g, bn2_b, num_inside):
    B = feat0.shape[0]
    pos = np.asarray(zone_mask[:, :, 0])
    order = np.argsort(~pos, axis=1, kind="stable")
    assert np.array_equal(
        order[:, :num_inside],
        np.broadcast_to(np.arange(num_inside), (B, num_inside)),
    ), "kernel assumes inside positions are the first num_inside rows"
    assert num_inside == NI

    bf = ml_dtypes.bfloat16
    f32 = np.float32

    def wt(w):  # [dout, din] -> [128, 2, dout]: [p, ki, o] = w[o, ki*128+p]
        return np.ascontiguousarray(
            w.T.reshape(2, 128, D).transpose(1, 0, 2)
        ).astype(bf)

    def cw(w, nchunk):  # [O, I, 3, 3] -> [128, nchunk*9, O], slot = c*9+k
        o_, i_, _, _ = w.shape
        r = w.transpose(1, 2, 3, 0).reshape(nchunk, 128, 9, o_)
        return np.ascontiguousarray(
            r.transpose(1, 0, 2, 3).reshape(128, nchunk * 9, o_)
        ).astype(bf)

    wk = wt(np.asarray(w_k, f32))
    wv = wt(np.asarray(w_v, f32))
    common = {
        "wqt": wt(np.asarray(w_q, f32)),
        "wkv": np.ascontiguousarray(np.concatenate([wk, wv], axis=2)),
        "c1w": cw(np.asarray(conv1_w, f32), 4),
        "c2w": cw(np.asarray(conv2_w, f32), 2),
        "bn1g": np.asarray(bn1_g, f32).reshape(D, 1),
        "bn1b": np.asarray(bn1_b, f32).reshape(D, 1),
        "bn2g": np.asarray(bn2_g, f32).reshape(D, 1),
        "bn2b": np.asarray(bn2_b, f32).reshape(D, 1),
        "mblk": _mblk(),
    }
    in_maps = []
    for b in range(NCORES):
        ft = np.asarray(feat0[b], f32).T.reshape(2, 128, H, W)  # [m, p, H, W]
        pad = np.zeros((128, 2, 84, PW), f32)
        pad[:, 0, 2:82, 1:81] = ft[0]
        pad[:, 1, 2:82, 1:81] = ft[1]
        m = dict(common)
        m["ip1f"] = pad.astype(bf)
        in_maps.append(m)
    return in_maps


def kernel(feat0, zone_mask, w_q, w_k, w_v, conv1_w, bn1_g, bn1_b,
           conv2_w, bn2_g, bn2_b, H=80, W=80, B=8, D=256, num_inside=3200,
           **_ignored):
    global LAST_EXEC_NS, LAST_MEAN_EXEC_NS
    if "nc" not in _cache:
        _cache["nc"] = build_nc()
    nc = _cache["nc"]

    in_maps = _prep_inputs(feat0, zone_mask, w_q, w_k, w_v, conv1_w, bn1_g,
                           bn1_b, conv2_w, bn2_g, bn2_b, int(num_inside))
    trace = os.environ.get("KERNEL_TRACE", "0") == "1"
    res = run_bass_kernel_spmd(nc, in_maps, list(range(NCORES)), trace=trace)
    LAST_EXEC_NS = res.exec_time_ns
    LAST_MEAN_EXEC_NS = res.mean_exec_time_ns
    out = np.empty((NCORES, HW, 256), np.float32)
    for b in range(NCORES):
        out[b] = res.results[b]["out_t"].T
    return out


# revision 27
# speedup vs baseline: 1.0379x; 1.0379x over previous
"""Trainium2 Bass kernel for LoFTR-style encoder layer (sparse attention + convs).

Sharding: pure data-parallel over batch B=8 -> 8 NeuronCores (one batch
element per core). BN statistics are all-reduced across cores.

Device layout is channel-major; the host pre-pads/casts feat0 to bf16
[128, 2, 84, 82] (conv-padded, image at rows 2..81, cols 1..80) so the
kernel DMAs straight into the conv1 input tiles.  K/V projections read
128-position blocks of the padded flat layout; pad positions project to
zero, giving K rows of exactly 1.0 (elu(0)+1) whose only side effect is
a constant +80 on Ksum, subtracted when building the block-diag matrix.

Emission follows the dependency chain (K/V proj -> KV -> Q -> message ->
conv1 -> BN1 -> conv2 -> BN2 -> out); the tile scheduler fills attention
stalls with independent conv1 matmuls (separate PSUM pools per concern).
"""

import os
import sys

import numpy as np

for _p in ("/opt/trn_rl_repo", os.path.expanduser("~/.axon_site/_ro/trn_rl_repo")):
    if os.path.isdir(_p) and _p not in sys.path:
        sys.path.insert(0, _p)

import ml_dtypes

import concourse.bass as bass
import concourse.mybir as mybir
import concourse.tile as tile
from concourse import bacc
from concourse.bass_utils import run_bass_kernel_spmd

F32 = mybir.dt.float32
BF16 = mybir.dt.bfloat16
AF = mybir.ActivationFunctionType
ALU = mybir.AluOpType

NCORES = 8
H = W = 80
HW = H * W          # 6400
D = 256
NI = 3200           # inside positions (image rows 0..39)
NHEAD = 8
PW = W + 2          # 82 padded width
ATTN_EPS = 1e-6
BN_EPS = 1e-5
BN_N = float(NCORES * HW)

# conv row-tiling: 5 output rows per psum tile -> N = 5*82 = 410 <= 512
RT = 5
NRT = H // RT       # 16
NT = RT * PW        # 410

# inside region in padded-flat coords: tile rows 2..41 -> [164, 3444)
IN0 = 2 * PW        # 164
NKV = 26            # ceil(3280 / 128); last tile is 80 wide
NPAD_IN = 80        # pad slots inside [164, 3444): 2 per row * 40 rows

# remote-dma 1-hop allreduce: correct by construction but the build-time
# scheduling sim is single-core and deadlocks on the remote semaphores,
# so it stays disabled until the sim models cross-core increments
FAST_AR = os.environ.get("FAST_AR", "0") == "1"

LAST_EXEC_NS = None
LAST_MEAN_EXEC_NS = None

_cache = {}


def _bd(ap3):
    return ap3.rearrange("p a b -> p (a b)")


def _r3(ap2, a):
    return ap2.rearrange("p (a b) -> p a b", a=a)


def build_nc():
    nc = bacc.Bacc(
        "TRN2", target_bir_lowering=False, debug=False, num_devices=NCORES
    )

    ip1_d = nc.dram_tensor("ip1f", [128, 2, 84, PW], BF16, kind="ExternalInput")
    wqt_d = nc.dram_tensor("wqt", [128, 2, D], BF16, kind="ExternalInput")
    wkv_d = nc.dram_tensor("wkv", [128, 2, 2 * D], BF16, kind="ExternalInput")
    c1w_d = nc.dram_tensor("c1w", [128, 36, D], BF16, kind="ExternalInput")
    c2w_d = nc.dram_tensor("c2w", [128, 18, D], BF16, kind="ExternalInput")
    bn1g_d = nc.dram_tensor("bn1g", [D, 1], F32, kind="ExternalInput")
    bn1b_d = nc.dram_tensor("bn1b", [D, 1], F32, kind="ExternalInput")
    bn2g_d = nc.dram_tensor("bn2g", [D, 1], F32, kind="ExternalInput")
    bn2b_d = nc.dram_tensor("bn2b", [D, 1], F32, kind="ExternalInput")
    mblk_d = nc.dram_tensor("mblk", [8, 256], BF16, kind="ExternalInput")
    out_d = nc.dram_tensor("out_t", [D, HW], BF16, kind="ExternalOutput")

    groups = [list(range(NCORES))]

    with tile.TileContext(nc) as tc:
        with (
            tc.tile_pool(name="pers", bufs=1) as pers,
            tc.tile_pool(name="bigp", bufs=2) as bigp,
            tc.tile_pool(name="qtp", bufs=1) as qtp,
            tc.tile_pool(name="scr", bufs=7) as scr,
            tc.tile_pool(name="sqp", bufs=3) as sqp,
            tc.tile_pool(name="rsp", bufs=5) as rsp,
            tc.tile_pool(name="small", bufs=1) as small,
            tc.tile_pool(name="fin", bufs=5) as fin,
            tc.tile_pool(name="sqF", bufs=2) as sqF,
            tc.tile_pool(name="psA", bufs=2, space="PSUM") as psA,
            tc.tile_pool(name="psK", bufs=2, space="PSUM") as psK,
            tc.tile_pool(name="psC", bufs=2, space="PSUM") as psC,
            tc.tile_pool(name="psF", bufs=2, space="PSUM") as psF,
            tc.tile_pool(name="dram", bufs=1, space="DRAM") as dramp,
        ):
            if FAST_AR:
                ar_rsem = nc.alloc_semaphore("ar_rsem")
                ar_lsem = nc.alloc_semaphore("ar_lsem")
                gbuf = [
                    pers.tile([128, 8, 4], F32, tag=f"gbuf{t}", name=f"gbuf{t}")
                    for t in range(2)
                ]
                for t in range(2):
                    nc.vector.memset(gbuf[t][:, :, :], 0.0)
                nc.gpsimd.bir_kernel_barrier_wait(groups)
            # ---------------- weights / inputs (chunked for early start) ----
            wqt = pers.tile([128, 2, D], BF16, tag="wqt", name="wqt")
            wkv = pers.tile([128, 2, 2 * D], BF16, tag="wkv", name="wkv")
            c1w = pers.tile([128, 36, D], BF16, tag="c1w", name="c1w")
            c2w = pers.tile([128, 18, D], BF16, tag="c2w", name="c2w")
            ip1 = [
                pers.tile([128, 84, PW], BF16, tag=f"ip1_{c}", name=f"ip1_{c}")
                for c in range(4)
            ]
            ip2 = [
                pers.tile([128, 84, PW], BF16, tag=f"ip2_{c}", name=f"ip2_{c}")
                for c in range(2)
            ]
            maskblk = pers.tile([8, 256], BF16, tag="maskblk", name="maskblk")

            # minimal first transfers: the very first K/V matmul needs only
            # input rows 0..3 of both halves plus the K|V weights
            for m in range(2):
                nc.sync.dma_start(ip1[m][:, 0:4, :], ip1_d[:, m, 0:4, :])
            nc.sync.dma_start(wkv[:, :, :], wkv_d[:, :, :])
            for r0, r1 in ((4, 7), (7, 21), (21, 42), (42, 63), (63, 84)):
                for m in range(2):
                    nc.sync.dma_start(
                        ip1[m][:, r0:r1, :], ip1_d[:, m, r0:r1, :]
                    )
                if r0 == 4:
                    nc.sync.dma_start(c1w[:, 0:9, :], c1w_d[:, 0:9, :])
                    nc.sync.dma_start(c1w[:, 9:18, :], c1w_d[:, 9:18, :])
                if r0 == 21:
                    nc.sync.dma_start(c1w[:, 18:36, :], c1w_d[:, 18:36, :])
                if r0 == 42:
                    nc.sync.dma_start(wqt[:, :, :], wqt_d[:, :, :])
            nc.sync.dma_start(c2w[:, :, :], c2w_d[:, :, :])
            nc.sync.dma_start(maskblk[:, :], mblk_d[:, :])

            eps_t = small.tile([128, 1], F32, tag="eps_t", name="eps_t")
            nc.vector.memset(eps_t[:, :], BN_EPS)
            g1 = small.tile([128, 2], F32, tag="g1", name="g1")
            b1 = small.tile([128, 2], F32, tag="b1", name="b1")
            g2 = small.tile([128, 2], F32, tag="g2", name="g2")
            b2 = small.tile([128, 2], F32, tag="b2", name="b2")
            for o in range(2):
                sl = slice(o * 128, (o + 1) * 128)
                nc.sync.dma_start(g1[:, o : o + 1], bn1g_d[sl, :])
                nc.sync.dma_start(b1[:, o : o + 1], bn1b_d[sl, :])
                nc.sync.dma_start(g2[:, o : o + 1], bn2g_d[sl, :])
                nc.sync.dma_start(b2[:, o : o + 1], bn2b_d[sl, :])

            # ---------------- zero-init --------------------------------------
            # (gpsimd sits behind the prelude barrier when FAST_AR, so early
            #  needs go on DVE; ip2 pads aren't read until conv2)
            ke = bigp.tile([128, NKV, D], BF16, tag="big", name="ke")
            ve = bigp.tile([128, NKV, D + 1], BF16, tag="big", name="ve")
            nc.vector.memset(ve[:, :, :], 1.0)  # col 0 of each chunk = ones
            for c in (2, 3):
                # t-channels: zero everything except where message is scattered
                nc.vector.memset(ip1[c][:, 0:42, :], 0.0)
                nc.vector.memset(ip1[c][:, 82:84, :], 0.0)
                nc.vector.memset(ip1[c][:, 42:82, 0:1], 0.0)
                nc.vector.memset(ip1[c][:, 42:82, 81:82], 0.0)
            for c in range(2):
                # ip2 pads (interior is fully written by conv1)
                nc.vector.memset(ip2[c][:, 0:2, :], 0.0)
                nc.vector.memset(ip2[c][:, 82:84, :], 0.0)
                nc.vector.memset(ip2[c][:, 2:82, 0:1], 0.0)
                nc.vector.memset(ip2[c][:, 2:82, 81:82], 0.0)

            ip1f = [_bd(ip1[c][:, :, :]) for c in range(4)]
            ip2f = [_bd(ip2[c][:, :, :]) for c in range(2)]

            # ---------------- K / V projections ([s, c] layout) -------------
            # one packed matmul per (i, ki): [128pos, 512] = [K | V]
            for i in range(NKV):
                mw = 128 if i < NKV - 1 else 80
                ps = psA.tile([128, 2 * D], F32, tag="psA", name="psA")
                for ki in range(2):
                    nc.tensor.matmul(
                        ps[:mw, :],
                        ip1f[ki][:, IN0 + i * 128 : IN0 + i * 128 + mw],
                        wkv[:, ki, :],
                        start=(ki == 0),
                        stop=(ki == 1),
                    )
                # elu(x)+1 = relu(x) + exp(min(x,0))
                sm = scr.tile([128, 2 * D], F32, tag="scr", name="sm")
                se = scr.tile([128, 2 * D], F32, tag="scr", name="se")
                nc.vector.tensor_scalar_min(sm[:mw, :D], ps[:mw, :D], 0.0)
                nc.scalar.activation(se[:mw, :D], sm[:mw, :D], AF.Exp)
                nc.vector.scalar_tensor_tensor(
                    ke[:mw, i, :], ps[:mw, :D], 0.0, se[:mw, :D], ALU.max, ALU.add
                )
                nc.scalar.copy(ve[:mw, i, 1:], ps[:mw, D:])

            # ---------------- KV + Ksum -> block-diag BD --------------------
            bd = [
                pers.tile([128, 264], BF16, tag=f"bd{m}", name=f"bd{m}")
                for m in range(2)
            ]
            for m in range(2):
                psm = psK.tile([128, D + 1], F32, tag="psK", name="psK")
                for i in range(NKV):
                    mw = 128 if i < NKV - 1 else 80
                    nc.tensor.matmul(
                        psm[:, :],
                        ke[:mw, i, m * 128 : (m + 1) * 128],
                        ve[:mw, i, :],
                        start=(i == 0),
                        stop=(i == NKV - 1),
                    )
                nc.vector.memset(bd[m][:, :], 0.0)
                for hh in range(4):
                    h = m * 4 + hh
                    lh = hh * 32
                    nc.vector.tensor_copy(
                        bd[m][lh : lh + 32, h * 32 : (h + 1) * 32],
                        psm[lh : lh + 32, 1 + h * 32 : 1 + (h + 1) * 32],
                    )
                    # pad positions add exactly NPAD_IN ones to Ksum
                    nc.vector.tensor_scalar_add(
                        bd[m][lh : lh + 32, 256 + h : 257 + h],
                        psm[lh : lh + 32, 0:1],
                        -float(NPAD_IN),
                    )

            # ---------------- Q projections + elu (all l-tiles) -------------
            qt = [
                [
                    qtp.tile([128, NT], BF16, tag=f"qt{q}_{m}", name=f"qt{q}_{m}")
                    for m in range(2)
                ]
                for q in range(8)
            ]
            # per q: projections, elu, S = Ksum.Q, Z = 1/(S+eps) — fused so the
            # Z chain's DVE ops sit right behind the elu in queue order
            rs = []

            def q_stage(q):
                lr = 42 + 5 * q
                for m in range(2):
                    ps = psA.tile([128, 2 * D], F32, tag="psA", name="psA")
                    for ki in range(2):
                        nc.tensor.matmul(
                            ps[:, :NT],
                            wqt[:, ki, m * 128 : (m + 1) * 128],
                            ip1f[ki][:, lr * PW : lr * PW + NT],
                            start=(ki == 0),
                            stop=(ki == 1),
                        )
                    sm = scr.tile([128, 2 * D], F32, tag="scr", name="smq")
                    se = scr.tile([128, 2 * D], F32, tag="scr", name="seq")
                    nc.vector.tensor_scalar_min(sm[:, :NT], ps[:, :NT], 0.0)
                    nc.scalar.activation(se[:, :NT], sm[:, :NT], AF.Exp)
                    nc.vector.scalar_tensor_tensor(
                        qt[q][m][:, :], ps[:, :NT], 0.0, se[:, :NT],
                        ALU.max, ALU.add,
                    )
                pss = psK.tile([8, NT], F32, tag="psK", name="psS")
                for ki in range(2):
                    nc.tensor.matmul(
                        pss[:, :],
                        bd[ki][:, 256:264],
                        qt[q][ki][:, :],
                        start=(ki == 0),
                        stop=(ki == 1),
                    )
                sadd = rsp.tile([8, NT], F32, tag="sadd", name="sadd")
                r = rsp.tile([8, NT], BF16, tag="rs", name="rs")
                nc.vector.tensor_scalar_add(sadd[:, :], pss[:, :], ATTN_EPS)
                with nc.allow_low_precision(reason="Z recip feeds bf16 conv"):
                    nc.vector.reciprocal(r[:, :], sadd[:, :])
                rs.append(r)

            q_stage(0)
            q_stage(1)

            # ---------- conv1 helper ----------------------------------------
            stats1 = small.tile([128, 4 * NRT], F32, tag="stats1", name="stats1")
            stats2 = small.tile([128, 4 * NRT], F32, tag="stats2", name="stats2")

            def conv1_tile(j, pool, ptag, sqpool, sqtag):
                r0 = RT * j
                for o in range(2):
                    taps = [(c, k) for c in range(2) for k in range(9)]
                    for c in (2, 3):
                        for ky in range(3):
                            if ky < 37 - r0:
                                continue  # t rows all zero in this stream
                            for kx in range(3):
                                taps.append((c, ky * 3 + kx))
                    ps = pool.tile([128, NT], F32, tag=ptag, name="psc1")
                    for idx, (c, k) in enumerate(taps):
                        ky, kx = divmod(k, 3)
                        s = (r0 + ky + 1) * PW + kx - 1
                        nc.tensor.matmul(
                            ps[:, :],
                            c1w[:, c * 9 + k, o * 128 : (o + 1) * 128],
                            ip1f[c][:, s : s + NT],
                            start=(idx == 0),
                            stop=(idx == len(taps) - 1),
                        )
                    val = _r3(ps[:, :], RT)[:, :, 1:81]
                    # copy raw y1 into ip2 interior + position-sum, on ACT
                    nc.scalar.activation(
                        ip2[o][:, r0 + 2 : r0 + 7, 1:81], val, AF.Identity,
                        accum_out=stats1[:, o * NRT + j : o * NRT + j + 1],
                    )
                    sq = sqpool.tile([128, NT], BF16, tag=sqtag, name="sq")
                    nc.scalar.activation(
                        _r3(sq[:, :], RT)[:, :, 1:81], val, AF.Square,
                        accum_out=stats1[:, (2 + o) * NRT + j :
                                         (2 + o) * NRT + j + 1],
                    )

            # ---------------- message + scatter, then dependent conv1 -------
            for q in range(8):
                if q + 2 < 8:
                    q_stage(q + 2)
                for m in range(2):
                    psg = psA.tile([128, 2 * D], F32, tag="psA", name="psA")
                    for ki in range(2):
                        nc.tensor.matmul(
                            psg[:, :NT],
                            bd[ki][:, m * 128 : (m + 1) * 128],
                            qt[q][ki][:, :],
                            start=(ki == 0),
                            stop=(ki == 1),
                        )
                    pre = psA.tile([128, 2 * D], F32, tag="psA", name="psA")
                    nc.tensor.matmul(
                        pre[:, :NT], maskblk[:, m * 128 : (m + 1) * 128],
                        rs[q][:, :],
                    )
                    preb = scr.tile([128, 2 * D], F32, tag="scr", name="preb")
                    nc.vector.tensor_copy(preb[:, :NT], pre[:, :NT])
                    nc.vector.tensor_tensor(
                        ip1[2 + m][:, 42 + 5 * q : 47 + 5 * q, 1:81],
                        _r3(psg[:, :NT], RT)[:, :, 1:81],
                        _r3(preb[:, :NT], RT)[:, :, 1:81],
                        ALU.mult,
                    )
                pl, pt = (psC, "psC") if q % 2 == 0 else (psK, "psK")
                conv1_tile(q + 7, pl, pt, sqp, "sq")
            conv1_tile(15, psC, "psC", sqp, "sq")

            # ---------------- independent conv1 (f-channels only) -----------
            # emitted last in the phase (lowest priority) with private PSUM /
            # scratch pools: the scheduler pulls these forward into every
            # attention stall without rotation edges back into the msg chain
            for j in range(7):
                conv1_tile(j, psF, "psF", sqF, "sqF")

            # ---------------- BN allreduce (2KB, latency-critical) ----------
            ar_round = [0]

            def bn_allreduce(stats, tag):
                bnst = small.tile([128, 4], F32, tag=f"bnst{tag}", name=f"bnst{tag}")
                nc.vector.tensor_reduce(
                    bnst[:, :], stats[:, :].rearrange("p (k j) -> p k j", j=NRT),
                    mybir.AxisListType.X, ALU.add,
                )
                gst = small.tile([128, 4], F32, tag=f"gst{tag}", name=f"gst{tag}")
                if FAST_AR:
                    # 1-hop all-to-all over the on-chip torus: XOR-distance i
                    # lands in gather slot i on every core (SPMD-safe), then a
                    # local tree-add.  remote_sem += 2 per arrived transfer.
                    rnd = ar_round[0]
                    ar_round[0] += 1
                    gb = gbuf[rnd]
                    for dist in range(8):
                        rd = [None] * 8
                        rd[dist] = (0, dist)
                        nc.gpsimd.remote_dma_broadcast(
                            gb[:, dist, :], bnst[:, :], ar_rsem, ar_lsem,
                            rdests=rd,
                        )
                    nc.gpsimd.trigger_dma(count=None)
                    nc.vector.wait_ge(ar_rsem, 16 * (rnd + 1))
                    h1 = small.tile([128, 4, 4], F32, tag=f"arh1{tag}",
                                    name=f"arh1{tag}")
                    h2 = small.tile([128, 2, 4], F32, tag=f"arh2{tag}",
                                    name=f"arh2{tag}")
                    nc.vector.tensor_tensor(
                        h1[:, :, :], gb[:, 0:4, :], gb[:, 4:8, :], ALU.add
                    )
                    nc.vector.tensor_tensor(
                        h2[:, :, :], h1[:, 0:2, :], h1[:, 2:4, :], ALU.add
                    )
                    nc.vector.tensor_tensor(
                        gst[:, :], h2[:, 0, :], h2[:, 1, :], ALU.add
                    )
                    return gst
                arin = dramp.tile([128, 4], F32, tag=f"arin{tag}", name=f"arin{tag}")
                arout = dramp.tile(
                    [NCORES * 128, 4], F32, tag=f"arout{tag}", name=f"arout{tag}"
                )
                nc.sync.dma_start(arin[:, :], bnst[:, :])
                # AllGather (one ring phase) + local tree-add beats AllReduce
                # (reduce-scatter + gather) for a latency-bound 2KB payload
                nc.gpsimd.collective_compute(
                    "AllGather", ALU.bypass, replica_groups=groups,
                    ins=[arin[:, :].opt()], outs=[arout[:, :].opt()],
                )
                gath = small.tile([128, 8, 4], F32, tag=f"gath{tag}",
                                  name=f"gath{tag}")
                nc.sync.dma_start(
                    gath[:, :, :],
                    arout[:, :].rearrange("(c p) f -> p c f", c=NCORES),
                )
                h1 = small.tile([128, 4, 4], F32, tag=f"arh1{tag}",
                                name=f"arh1{tag}")
                h2 = small.tile([128, 2, 4], F32, tag=f"arh2{tag}",
                                name=f"arh2{tag}")
                nc.vector.tensor_tensor(
                    h1[:, :, :], gath[:, 0:4, :], gath[:, 4:8, :], ALU.add
                )
                nc.vector.tensor_tensor(
                    h2[:, :, :], h1[:, 0:2, :], h1[:, 2:4, :], ALU.add
                )
                nc.vector.tensor_tensor(
                    gst[:, :], h2[:, 0, :], h2[:, 1, :], ALU.add
                )
                return gst

            def bn_coeffs(gst, gg, bb, tag):
                # gst = [sum_o0, sum_o1, sq_o0, sq_o1] -> both halves at once
                nm = small.tile([128, 2], F32, tag=f"nm{tag}", name=f"nm{tag}")
                ex2 = small.tile([128, 2], F32, tag=f"ex2{tag}", name=f"ex2{tag}")
                m2 = small.tile([128, 2], F32, tag=f"m2{tag}", name=f"m2{tag}")
                var = small.tile([128, 2], F32, tag=f"var{tag}", name=f"var{tag}")
                sd = small.tile([128, 2], F32, tag=f"sd{tag}", name=f"sd{tag}")
                rsd = small.tile([128, 2], F32, tag=f"rsd{tag}", name=f"rsd{tag}")
                scl = small.tile([128, 2], F32, tag=f"scl{tag}", name=f"scl{tag}")
                sht = small.tile([128, 2], F32, tag=f"sht{tag}", name=f"sht{tag}")
                sh = small.tile([128, 2], F32, tag=f"sh{tag}", name=f"sh{tag}")
                nc.vector.tensor_scalar_mul(nm[:, :], gst[:, 0:2], -1.0 / BN_N)
                nc.vector.tensor_scalar_mul(ex2[:, :], gst[:, 2:4], 1.0 / BN_N)
                # var_neg = m^2 - E[x^2];  sd = sqrt(-var_neg + eps)
                nc.vector.tensor_tensor(m2[:, :], nm[:, :], nm[:, :], ALU.mult)
                nc.vector.tensor_tensor(
                    var[:, :], m2[:, :], ex2[:, :], ALU.subtract
                )
                nc.scalar.activation(
                    sd[:, :], var[:, :], AF.Sqrt, bias=eps_t[:, 0:1], scale=-1.0
                )
                nc.vector.reciprocal(rsd[:, :], sd[:, :])
                nc.vector.tensor_tensor(scl[:, :], rsd[:, :], gg[:, :], ALU.mult)
                nc.vector.tensor_tensor(sht[:, :], nm[:, :], scl[:, :], ALU.mult)
                nc.vector.tensor_tensor(sh[:, :], sht[:, :], bb[:, :], ALU.add)
                return scl, sh

            gst1 = bn_allreduce(stats1, "1")
            scl1, sh1 = bn_coeffs(gst1, g1, b1, "1")

            # ---------------- normalize (in-place) pipelined with conv2 -----
            y2 = [
                bigp.tile([128, H, W], BF16, tag="big", name=f"y2_{o}")
                for o in range(2)
            ]

            def norm_chunk(ra, rb):
                for o in range(2):
                    nc.vector.tensor_scalar(
                        ip2[o][:, ra:rb, 1:81],
                        ip2[o][:, ra:rb, 1:81],
                        scl1[:, o : o + 1],
                        sh1[:, o : o + 1],
                        ALU.mult,
                        ALU.add,
                    )

            def conv2_tile(j):
                r0 = RT * j
                for o in range(2):
                    ps = psC.tile([128, NT], F32, tag="psC", name="psC")
                    idx = 0
                    for c in range(2):
                        for k in range(9):
                            ky, kx = divmod(k, 3)
                            s = (r0 + ky + 1) * PW + kx - 1
                            nc.tensor.matmul(
                                ps[:, :],
                                c2w[:, c * 9 + k, o * 128 : (o + 1) * 128],
                                ip2f[c][:, s : s + NT],
                                start=(idx == 0),
                                stop=(idx == 17),
                            )
                            idx += 1
                    val = _r3(ps[:, :], RT)[:, :, 1:81]
                    nc.scalar.activation(
                        y2[o][:, r0 : r0 + 5, :], val, AF.Identity,
                        accum_out=stats2[:, o * NRT + j : o * NRT + j + 1],
                    )
                    sq = sqp.tile([128, NT], BF16, tag="sq", name="sq2")
                    nc.scalar.activation(
                        _r3(sq[:, :], RT)[:, :, 1:81], val, AF.Square,
                        accum_out=stats2[:, (2 + o) * NRT + j :
                                         (2 + o) * NRT + j + 1],
                    )

            norm_chunk(2, 8)
            conv2_tile(0)
            for k in range(1, 8):
                norm_chunk(10 * k - 2, 10 * k + 8)
                conv2_tile(2 * k - 1)
                conv2_tile(2 * k)
            norm_chunk(78, 82)
            conv2_tile(15)

            # ---------------- BN2 allreduce + residual + store --------------
            gst2 = bn_allreduce(stats2, "2")
            scl2, sh2 = bn_coeffs(gst2, g2, b2, "2")

            for o in range(2):
                for k in range(8):
                    n = o * 8 + k
                    fsl = slice(800 * k, 800 * (k + 1))
                    tmp = fin.tile([128, 800], BF16, tag="tmp", name="tmp")
                    nc.scalar.activation(
                        _r3(tmp[:, :], 10),
                        y2[o][:, 10 * k : 10 * (k + 1), :],
                        AF.Identity,
                        bias=sh2[:, o : o + 1],
                        scale=scl2[:, o : o + 1],
                    )
                    eng = nc.gpsimd if n % 3 == 2 else nc.vector
                    eng.tensor_tensor(
                        _r3(tmp[:, :], 10),
                        _r3(tmp[:, :], 10),
                        ip1[o][:, 2 + 10 * k : 12 + 10 * k, 1:81],
                        ALU.add,
                    )
                    nc.sync.dma_start(out_d[o * 128 : (o + 1) * 128, fsl], tmp[:, :])

    nc.compile()
    return nc


def _mblk():
    mb = np.zeros((8, 256), np.float32)
    for h in range(8):
        mb[h, h * 32 : (h + 1) * 32] = 1.0
    return mb.astype(ml_dtypes.bfloat16)


def _prep_inputs(feat0, zone_mask, w_q, w_k, w_v, conv1_w, bn1_g, bn1_b,
                 conv2_w, bn2_g, bn2_b, num_inside):
    B = feat0.shape[0]
    pos = np.asarray(zone_mask[:, :, 0])
    order = np.argsort(~pos, axis=1, kind="stable")
    assert np.array_equal(
        order[:, :num_inside],
        np.broadcast_to(np.arange(num_inside), (B, num_inside)),
    ), "kernel assumes inside positions are the first num_inside rows"
    assert num_inside == NI

    bf = ml_dtypes.bfloat16
    f32 = np.float32

    def wt(w):  # [dout, din] -> [128, 2, dout]: [p, ki, o] = w[o, ki*128+p]
        return np.ascontiguousarray(
            w.T.reshape(2, 128, D).transpose(1, 0, 2)
        ).astype(bf)

    def cw(w, nchunk):  # [O, I, 3, 3] -> [128, nchunk*9, O], slot = c*9+k
        o_, i_, _, _ = w.shape
        r = w.transpose(1, 2, 3, 0).reshape(nchunk, 128, 9, o_)
        return np.ascontiguousarray(
            r.transpose(1, 0, 2, 3).reshape(128, nchunk * 9, o_)
        ).astype(bf)

    wk = wt(np.asarray(w_k, f32))
    wv = wt(np.asarray(w_v, f32))
    common = {
        "wqt": wt(np.asarray(w_q, f32)),
        "wkv": np.ascontiguousarray(np.concatenate([wk, wv], axis=2)),
        "c1w": cw(np.asarray(conv1_w, f32), 4),
        "c2w": cw(np.asarray(conv2_w, f32), 2),
        "bn1g": np.asarray(bn1_g, f32).reshape(D, 1),
        "bn1b": np.asarray(bn1_b, f32).reshape(D, 1),
        "bn2g": np.asarray(bn2_g, f32).reshape(D, 1),
        "bn2b": np.asarray(bn2_b, f32).reshape(D, 1),
        "mblk": _mblk(),
    }
    in_maps = []
    for b in range(NCORES):
        ft = np.asarray(feat0[b], f32).T.reshape(2, 128, H, W)  # [m, p, H, W]
        pad = np.zeros((128, 2, 84, PW), f32)
        pad[:, 0, 2:82, 1:81] = ft[0]
        pad[:, 1, 2:82, 1:81] = ft[1]
        m = dict(common)
        m["ip1f"] = pad.astype(bf)
        in_maps.append(m)
    return in_maps


def kernel(feat0, zone_mask, w_q, w_k, w_v, conv1_w, bn1_g, bn1_b,
           conv2_w, bn2_g, bn2_b, H=80, W=80, B=8, D=256, num_inside=3200,
           **_ignored):
    global LAST_EXEC_NS, LAST_MEAN_EXEC_NS
    if "nc" not in _cache:
        _cache["nc"] = build_nc()
    nc = _cache["nc"]

    in_maps = _prep_inputs(feat0, zone_mask, w_q, w_k, w_v, conv1_w, bn1_g,
                           bn1_b, conv2_w, bn2_g, bn2_b, int(num_inside))
    trace = os.environ.get("KERNEL_TRACE", "0") == "1"
    res = run_bass_kernel_spmd(nc, in_maps, list(range(NCORES)), trace=trace)
    LAST_EXEC_NS = res.exec_time_ns
    LAST_MEAN_EXEC_NS = res.mean_exec_time_ns
    out = np.empty((NCORES, HW, 256), np.float32)
    for b in range(NCORES):
        out[b] = np.asarray(res.results[b]["out_t"], dtype=np.float32).T
    return out
